# revision 1
# baseline (speedup 1.0000x reference)
"""Trainium2 Bass kernel for NT-Xent contrastive loss (BATCH=4096, DIM=512, TEMP=0.5).

Strategy (data-parallel over rows of the 2B x 2B similarity matrix):
  - Host: E = concat(emb_i, emb_j) [8192, 512] f32, cast bf16. Each core gets
    ET = E.T (replicated) + its own 1024-column block, plus row-major copies
    (full + own + partner) pre-tiled into the SBUF image layout, an identity
    and a row-selector constant.
  - Device (per core, SPMD, no collectives):
      * sumsq of every row via DVE scalar_tensor_tensor square+accumulate
      * r = 1/||e|| = exp(-0.5*ln(sumsq)) on ACT -- Exp and Ln share one
        activation table set, so the kernel never swaps tables
      * broadcast r across partitions with PE: transpose r-block via the
        tensor engine, then one selector matmul per row-tile
      * normalize the rhs copy column-wise in place: z_j = e_j * r_j (DVE)
      * S' = e_block^T @ Z on PE (bf16, fp32 accum); first two column groups
        as [128,1024] PSUM groups (early start), rest as [128,2048] pairs
      * ACT: exp(S' * r_row/TEMP) with fused row-sum accumulation
      * positives via DVE row-dots of own x partner row-major blocks
      * per-core partial: sum_rows(log(den - e^{1/TEMP}) - pos/TEMP) -> [1,1]
  - Host: loss = sum(partials) / (2B).

Emission order is deliberate: per-engine queue order paces the normalization
pipeline (DMA -> DVE sumsq -> ACT ln/exp -> PE broadcast -> DVE normalize)
just ahead of the PE/ACT main-loop stream.
"""

import math

import ml_dtypes
import numpy as np

BATCH = 4096
DIM = 512
TEMP = 0.5
B2 = 2 * BATCH              # 8192 rows/cols of the similarity matrix
NCORES = 8
RPC = B2 // NCORES          # 1024 rows per core
KT = DIM // 128             # 4 contraction chunks
CG = 8                      # column groups
CGW = B2 // CG              # 1024 columns per group
T8 = RPC // 128             # 8 row-tiles per group / per core
NBF = CGW // 512            # 512-wide matmuls per group
NG = 5                      # main groups per row-tile: c0, c1, cp1, cp2, cp3
EXP_DIAG = math.exp(1.0 / TEMP)

_CACHE = {}


def _build():
    import concourse.bacc as bacc
    import concourse.mybir as mybir
    import concourse.tile as tile

    f32 = mybir.dt.float32
    bf16 = mybir.dt.bfloat16
    AF = mybir.ActivationFunctionType
    ALU = mybir.AluOpType
    X = mybir.AxisListType.X

    import bass_rust as _bass_rust
    from concourse.hw_specs import get_activation_tables

    class _Bacc(bacc.Bacc):
        """Bacc that pins Exp+Ln to the combined natural_log_exp_and_others
        activation-table set, so the kernel never swaps ACT tables."""

        def insert_act_table_loads(self):
            has_activation = any(
                isinstance(i, mybir.InstActivation)
                for b in self.main_func.blocks
                for i in b.instructions)
            if not has_activation:
                return
            drop = {mybir.ActivationFunctionType.Exp,
                    mybir.ActivationFunctionType.Ln}
            tables = []
            for name, funcs in get_activation_tables(self.m.arch).items():
                if name != "natural_log_exp_and_others":
                    funcs = funcs - drop
                tables.append((name, funcs))
            _bass_rust.insert_act_table_loads(self, tables)

    nc = _Bacc("TRN2", target_bir_lowering=False, debug=False,
               num_devices=NCORES)

    et_d = nc.dram_tensor("et", [DIM, B2], bf16, kind="ExternalInput").ap()
    etb_d = nc.dram_tensor("etb", [DIM, RPC], bf16, kind="ExternalInput").ap()
    erm_d = nc.dram_tensor("erm", [128, (B2 // 128) * DIM], bf16,
                           kind="ExternalInput").ap()
    ermb_d = nc.dram_tensor("ermb", [128, T8 * DIM], bf16,
                            kind="ExternalInput").ap()
    ermp_d = nc.dram_tensor("ermp", [128, T8 * DIM], bf16,
                            kind="ExternalInput").ap()
    iden_d = nc.dram_tensor("iden", [128, 128], bf16, kind="ExternalInput").ap()
    sel_d = nc.dram_tensor("sel", [128, T8 * 128], bf16,
                           kind="ExternalInput").ap()
    out_d = nc.dram_tensor("out", [1, 1], f32, kind="ExternalOutput").ap()

    with tile.TileContext(nc) as tc:
        with (
            tc.tile_pool(name="persist", bufs=1) as P,
            tc.tile_pool(name="scratch", bufs=2) as S,
            tc.tile_pool(name="psum", bufs=2, space="PSUM") as PS,
        ):
            ss64 = P.tile([128, 64], f32, name="ss64")
            ssb = P.tile([128, T8], f32, name="ssb")
            ssp = P.tile([128, T8], f32, name="ssp")
            rawpos = P.tile([128, T8], f32, name="rawpos")
            rsums = P.tile([128, T8 * NG], f32, name="rsums")
            sc8 = P.tile([128, T8], f32, name="sc8")
            pos8 = P.tile([128, T8], f32, name="pos8")
            ones = P.tile([128, 1], f32, name="ones")
            iden = P.tile([128, 128], bf16, name="iden")
            sel = P.tile([128, T8 * 128], bf16, name="sel")
            rbc = [P.tile([128, CGW], bf16, name=f"rbc_{c}") for c in range(CG)]
            erm = [None] * CG
            et2 = [[None] * CG for _ in range(KT)]   # raw (recycled scratch)
            etn = [[P.tile([128, CGW], bf16, name=f"etn_{k}_{c}")
                    for c in range(CG)] for k in range(KT)]
            etb = [None] * KT

            nc.vector.memset(ones[:], 1.0)
            nc.sync.dma_start(iden[:], iden_d[:])
            nc.sync.dma_start(sel[:], sel_d[:])

            def load_rm(dram_ap, name):
                sb = P.tile([128, T8 * DIM], bf16, name=name)
                nc.sync.dma_start(sb[:], dram_ap)
                return sb

            def load_erm(c):
                erm[c] = load_rm(erm_d[:, c * T8 * DIM:(c + 1) * T8 * DIM],
                                 f"erm_{c}")

            def load_et(c):
                for k in range(KT):
                    et2[k][c] = S.tile([128, CGW], bf16, name=f"et_{k}_{c}",
                                       tag="etraw", bufs=8)
                    nc.sync.dma_start(
                        et2[k][c][:],
                        et_d[k * 128:(k + 1) * 128, c * CGW:(c + 1) * CGW])

            def sumsq(src, tt, dst, dcol, src2=None):
                sco = S.tile([128, DIM], bf16, tag="stt", name="sco")
                s2 = src2 if src2 is not None else src
                nc.vector.scalar_tensor_tensor(
                    sco[:], src[:, tt * DIM:(tt + 1) * DIM], 1.0,
                    s2[:, tt * DIM:(tt + 1) * DIM], ALU.mult, ALU.mult,
                    accum_out=dst[:, dcol:dcol + 1])

            def rsqrt(dst, src_ap, w):
                """dst[:, 0:w] = 1/sqrt(src) via exp(-0.5*ln(x)) -- same ACT
                table set as the main-loop Exp, so no table swaps."""
                ln = S.tile([128, w], f32, tag=f"ln{w}", name="ln")
                nc.scalar.activation(ln[:], src_ap, AF.Ln)
                nc.scalar.activation(dst, ln[:], AF.Exp, scale=-0.5)

            def rchain(c):
                """r for group c -> broadcast via PE -> normalize in place."""
                rcb = S.tile([128, 128], bf16, tag="rcb", name="rcb")
                nc.vector.memset(rcb[:], 0.0)
                rsqrt(rcb[:, 0:T8], ss64[:, c * 8:(c + 1) * 8], T8)
                ptr = PS.tile([128, 128], bf16, tag="mm", name="ptr")
                nc.tensor.transpose(ptr[:], rcb[:], iden[:])
                rT = S.tile([128, 128], bf16, tag="rT", name="rT")
                nc.vector.tensor_copy(rT[:], ptr[:])
                pb = PS.tile([128, CGW], f32, tag="mm", name="pb")
                for t in range(T8):
                    nc.tensor.matmul(pb[:, t * 128:(t + 1) * 128],
                                     sel[:, t * 128:(t + 1) * 128],
                                     rT[:], start=True, stop=True)
                nc.vector.tensor_copy(rbc[c][:], pb[:])
                eng = nc.vector if c < 2 else nc.gpsimd
                for k in range(KT):
                    eng.tensor_tensor(etn[k][c][:], et2[k][c][:],
                                      rbc[c][:], ALU.mult)

            def main_group(gi, cgs):
                """One main group per row-tile over the given column groups."""
                for t in range(T8):
                    wid = len(cgs) * CGW
                    ps = PS.tile([128, wid], f32, tag="mm", name="psmm")
                    for k in range(KT):
                        for ci, c in enumerate(cgs):
                            for n in range(NBF):
                                lo = ci * CGW + n * 512
                                nc.tensor.matmul(
                                    ps[:, lo:lo + 512],
                                    etb[k][:, t * 128:(t + 1) * 128],
                                    etn[k][c][:, n * 512:(n + 1) * 512],
                                    start=(k == 0), stop=(k == KT - 1))
                    sce = S.tile([128, wid], bf16, tag="expout", name="sce")
                    col = t * NG + gi
                    nc.scalar.activation(sce[:], ps[:], AF.Exp,
                                         scale=sc8[:, t:t + 1],
                                         accum_out=rsums[:, col:col + 1])

            # ---- paced emission ----
            load_erm(0)
            ermb = load_rm(ermb_d[:, :], "ermb")
            load_erm(1)
            for k in range(KT):
                etb[k] = P.tile([128, RPC], bf16, name=f"etb_{k}")
                nc.sync.dma_start(etb[k][:], etb_d[k * 128:(k + 1) * 128, :])
            load_et(0)
            load_et(1)
            for tt in range(T8):
                sumsq(erm[0], tt, ss64, tt)
            rchain(0)
            for t in range(T8):                      # own norms
                sumsq(ermb, t, ssb, t)
            rb8 = P.tile([128, T8], f32, name="rb8")
            rsqrt(rb8[:], ssb[:], T8)
            nc.vector.tensor_scalar_mul(sc8[:], rb8[:], 1.0 / TEMP)
            main_group(0, (0,))

            load_erm(2)
            load_erm(3)
            load_et(2)
            load_et(3)
            for tt in range(T8):
                sumsq(erm[1], tt, ss64, 8 + tt)
            rchain(1)
            main_group(1, (1,))

            ermp = load_rm(ermp_d[:, :], "ermp")
            load_erm(4)
            load_erm(5)
            load_et(4)
            load_et(5)
            for c in (2, 3):
                for tt in range(T8):
                    sumsq(erm[c], tt, ss64, c * 8 + tt)
                rchain(c)
            main_group(2, (2, 3))

            load_erm(6)
            load_erm(7)
            load_et(6)
            load_et(7)
            for c in (4, 5):
                for tt in range(T8):
                    sumsq(erm[c], tt, ss64, c * 8 + tt)
                rchain(c)
            main_group(3, (4, 5))

            for c in (6, 7):
                for tt in range(T8):
                    sumsq(erm[c], tt, ss64, c * 8 + tt)
                rchain(c)
            for t in range(T8):                      # partner norms + positives
                sumsq(ermp, t, ssp, t)
            for t in range(T8):
                sumsq(ermb, t, rawpos, t, src2=ermp)
            rp8 = P.tile([128, T8], f32, name="rp8")
            rsqrt(rp8[:], ssp[:], T8)
            pt0 = P.tile([128, T8], f32, name="pt0")
            nc.vector.tensor_mul(pt0[:], rawpos[:], rb8[:])
            pt1 = P.tile([128, T8], f32, name="pt1")
            nc.vector.tensor_mul(pt1[:], pt0[:], rp8[:])
            nc.vector.tensor_scalar_mul(pos8[:], pt1[:], 1.0 / TEMP)

            main_group(4, (6, 7))

            # ---- finalize: den = rowsum - e^{1/T}; sum(log(den) - pos) ----
            den8 = P.tile([128, T8], f32, name="den8")
            nc.vector.tensor_reduce(
                den8[:], rsums[:].rearrange("p (t c) -> p t c", c=NG),
                X, ALU.add)
            den8b = P.tile([128, T8], f32, name="den8b")
            nc.vector.tensor_scalar_add(den8b[:], den8[:], -EXP_DIAG)
            logd = S.tile([128, T8], f32, tag="logd", name="logd")
            tlog = P.tile([128, 1], f32, name="tlog")
            nc.scalar.activation(logd[:], den8b[:], AF.Ln, accum_out=tlog[:])
            tpos = P.tile([128, 1], f32, name="tpos")
            nc.vector.tensor_reduce(tpos[:], pos8[:], X, ALU.add)
            lv = P.tile([128, 1], f32, name="lv")
            nc.vector.tensor_sub(lv[:], tlog[:], tpos[:])
            psf = PS.tile([1, 1], f32, tag="mm", name="psf")
            nc.tensor.matmul(psf[:], lv[:], ones[:], start=True, stop=True)
            ob = P.tile([1, 1], f32, name="ob")
            nc.vector.tensor_copy(ob[:], psf[:])
            nc.sync.dma_start(out_d[:], ob[:])

    nc.compile()
    return nc


def _get_nc():
    if "nc" not in _CACHE:
        _CACHE["nc"] = _build()
    return _CACHE["nc"]


def _in_maps(emb_i, emb_j):
    bf = ml_dtypes.bfloat16
    E = np.concatenate([np.asarray(emb_i, dtype=np.float32),
                        np.asarray(emb_j, dtype=np.float32)], axis=0)
    Ebf = E.astype(bf)                              # [8192, 512] row-major
    ET = np.ascontiguousarray(Ebf.T)                # [512, 8192]
    # SBUF-image tiling of the row-major copy: ERMT[p, t*512+d] = Ebf[t*128+p, d]
    ERMT = np.ascontiguousarray(
        Ebf.reshape(B2 // 128, 128, DIM).transpose(1, 0, 2).reshape(128, -1))
    SEL = np.zeros((128, T8 * 128), dtype=bf)
    for tp in range(T8):
        SEL[tp, tp * 128:(tp + 1) * 128] = 1.0
    maps = []
    for k in range(NCORES):
        s = k * RPC
        p = (s + BATCH) % B2
        maps.append({
            "et": ET,
            "etb": np.ascontiguousarray(ET[:, s:s + RPC]),
            "erm": ERMT,
            "ermb": np.ascontiguousarray(
                ERMT[:, s // 128 * DIM:(s // 128 + T8) * DIM]),
            "ermp": np.ascontiguousarray(
                ERMT[:, p // 128 * DIM:(p // 128 + T8) * DIM]),
            "iden": np.eye(128, dtype=bf),
            "sel": SEL,
        })
    return maps


def _run(emb_i, emb_j, trace=False):
    from concourse.bass_utils import run_bass_kernel_spmd
    nc = _get_nc()
    res = run_bass_kernel_spmd(nc, _in_maps(emb_i, emb_j),
                               list(range(NCORES)), trace=trace)
    total = sum(float(res.results[i]["out"][0, 0]) for i in range(NCORES))
    loss = np.float32(total / B2)
    return loss, res


def kernel(emb_i, emb_j):
    return _run(emb_i, emb_j, trace=False)[0]



# revision 5
# speedup vs baseline: 1.3154x; 1.3154x over previous
"""Trainium2 Bass kernel for NT-Xent contrastive loss (BATCH=4096, DIM=512, TEMP=0.5).

fp8 (DoubleRow) version of the data-parallel row-sharded design:
  - Host: E = concat(emb_i, emb_j) [8192, 512] f32 -> fp8e4 (TRN E4M3).
    Each core receives a column-ROTATED transpose image (own block first)
    plus the matching rotated row-major image, a 128x128 identity and an
    alpha-scaled row-selector constant.
  - Device (per core, SPMD, no collectives):
      * sumsq of every row via DVE/GpSimd scalar_tensor_tensor (fp8 in,
        f32 accum); r = 1/||e|| = exp(-0.5*ln(ss)) on ACT (Exp+Ln share
        one table set -> no table swaps)
      * broadcast alpha*r across partitions with PE (transpose + selector
        matmul), normalize the column image in place to fp8: zq = Eq*r*alpha
      * S' = zq_own^T @ zq on PE in fp8 perf_mode=DoubleRow (K packed 2x),
        PSUM f32; groups (0)(1,2)(3,4)(5,6)(7) per row-tile
      * ACT: exp(S'/(alpha^2*T)) with fused row-sum accumulation
      * positives via DVE row-dots of own x partner row-major blocks
      * per-core partial: sum_rows(log(den - e^{1/T}) - pos/T) -> [1,1]
  - Host: loss = sum(partials) / (2B).
"""

import math

import ml_dtypes
import numpy as np

BATCH = 4096
DIM = 512
TEMP = 0.5
B2 = 2 * BATCH              # 8192 rows/cols of the similarity matrix
NCORES = 8
RPC = B2 // NCORES          # 1024 rows per core
KT = DIM // 128             # 4 contraction chunks of 128
CG = 8                      # column groups
CGW = B2 // CG              # 1024 columns per group
T8 = RPC // 128             # 8 row-tiles per core
NG = 5                      # main groups per row-tile: (0)(12)(34)(56)(7)
ALPHA = 16.0                # fp8 scale for normalized operands
ASCALE = 1.0 / (ALPHA * ALPHA * TEMP)
EXP_DIAG = math.exp(1.0 / TEMP)

_CACHE = {}


def _build():
    import concourse.bacc as bacc
    import concourse.mybir as mybir
    import concourse.tile as tile

    f32 = mybir.dt.float32
    bf16 = mybir.dt.bfloat16
    fp8 = mybir.dt.float8e4
    AF = mybir.ActivationFunctionType
    ALU = mybir.AluOpType
    X = mybir.AxisListType.X
    DR = mybir.MatmulPerfMode.DoubleRow

    import bass_rust as _bass_rust
    from concourse.hw_specs import get_activation_tables

    class _Bacc(bacc.Bacc):
        """Bacc that pins Exp+Ln to the combined natural_log_exp_and_others
        activation-table set, so the kernel never swaps ACT tables."""

        def insert_act_table_loads(self):
            has_activation = any(
                isinstance(i, mybir.InstActivation)
                for b in self.main_func.blocks
                for i in b.instructions)
            if not has_activation:
                return
            drop = {mybir.ActivationFunctionType.Exp,
                    mybir.ActivationFunctionType.Ln}
            tables = []
            for name, funcs in get_activation_tables(self.m.arch).items():
                if name != "natural_log_exp_and_others":
                    funcs = funcs - drop
                tables.append((name, funcs))
            _bass_rust.insert_act_table_loads(self, tables)

    nc = _Bacc("TRN2", target_bir_lowering=False, debug=False,
               num_devices=NCORES)

    et_d = nc.dram_tensor("et", [DIM, B2], fp8, kind="ExternalInput").ap()
    erm_d = nc.dram_tensor("erm", [128, (B2 // 128) * DIM], fp8,
                           kind="ExternalInput").ap()
    iden_d = nc.dram_tensor("iden", [128, 128], bf16, kind="ExternalInput").ap()
    sel_d = nc.dram_tensor("sel", [128, T8 * 128], bf16,
                           kind="ExternalInput").ap()
    out_d = nc.dram_tensor("out", [1, 1], f32, kind="ExternalOutput").ap()

    with tile.TileContext(nc) as tc:
        with (
            tc.tile_pool(name="persist", bufs=1) as P,
            tc.tile_pool(name="scratch", bufs=2) as S,
            tc.tile_pool(name="psum", bufs=2, space="PSUM") as PS,
        ):
            ss64 = P.tile([128, CG * T8], f32, name="ss64")
            rawpos = P.tile([128, T8], f32, name="rawpos")
            rsums = P.tile([128, T8 * NG], f32, name="rsums")
            rb8 = P.tile([128, T8], f32, name="rb8")
            rp8 = P.tile([128, T8], f32, name="rp8")
            pos8 = P.tile([128, T8], f32, name="pos8")
            ones = P.tile([128, 1], f32, name="ones")
            iden = P.tile([128, 128], bf16, name="iden")
            sel = P.tile([128, T8 * 128], bf16, name="sel")
            erm = [P.tile([128, T8 * DIM], fp8, name=f"erm_{c}")
                   for c in range(CG)]
            etn = [P.tile([128, KT, CGW], fp8, name=f"etn_{c}")
                   for c in range(CG)]
            et3 = [None] * CG

            nc.vector.memset(ones[:], 1.0)
            nc.sync.dma_start(iden[:], iden_d[:])
            nc.sync.dma_start(sel[:], sel_d[:])

            def load_c(c):
                nc.sync.dma_start(erm[c][:], erm_d[:, c * T8 * DIM:
                                                   (c + 1) * T8 * DIM])
                et3[c] = S.tile([128, KT, CGW], fp8, name=f"et_{c}",
                                tag="etraw", bufs=3)
                for k in range(KT):
                    nc.sync.dma_start(
                        et3[c][:, k, :],
                        et_d[k * 128:(k + 1) * 128, c * CGW:(c + 1) * CGW])

            def sumsq(c):
                """sumsq of group-c rows into ss64[:, c*8:(c+1)*8] (DVE)."""
                for t in range(T8):
                    sco = S.tile([128, DIM], fp8, tag="sttv", name="sco")
                    src = erm[c][:, t * DIM:(t + 1) * DIM]
                    nc.vector.scalar_tensor_tensor(
                        sco[:], src, 1.0, src, ALU.mult, ALU.mult,
                        accum_out=ss64[:, c * T8 + t:c * T8 + t + 1])

            def rsqrt(dst_ap, src_ap, w):
                """dst = 1/sqrt(src) via exp(-0.5*ln(x)) on ACT."""
                ln = S.tile([128, w], f32, tag=f"ln{w}", name="ln")
                nc.scalar.activation(ln[:], src_ap, AF.Ln)
                nc.scalar.activation(dst_ap, ln[:], AF.Exp, scale=-0.5)

            def rchain(c, r8_src=None):
                """alpha*r for group c broadcast down partitions -> rbc,
                then normalize et3[c] -> etn[c] (fp8)."""
                rcb = S.tile([128, 128], bf16, tag="rcb", name="rcb")
                nc.vector.memset(rcb[:], 0.0)
                if r8_src is None:
                    rsqrt(rcb[:, 0:T8], ss64[:, c * T8:(c + 1) * T8], T8)
                else:
                    nc.vector.tensor_copy(rcb[:, 0:T8], r8_src)
                ptr = PS.tile([128, 128], bf16, tag="mm", name="ptr")
                nc.tensor.transpose(ptr[:], rcb[:], iden[:])
                rT = S.tile([128, 128], bf16, tag="rT", name="rT")
                nc.vector.tensor_copy(rT[:], ptr[:])
                pb = PS.tile([128, CGW], f32, tag="mm", name="pb")
                for t in range(T8):
                    nc.tensor.matmul(pb[:, t * 128:(t + 1) * 128],
                                     sel[:, t * 128:(t + 1) * 128],
                                     rT[:], start=True, stop=True)
                rbc = S.tile([128, CGW], f32, tag="rbc", name="rbc", bufs=2)
                nc.vector.tensor_copy(rbc[:], pb[:])
                for k in range(KT):
                    eng = nc.vector if c == 0 else nc.gpsimd
                    eng.tensor_tensor(etn[c][:, k, :], et3[c][:, k, :],
                                      rbc[:], ALU.mult)

            def main_group(gi, cgs):
                """Main fp8 DoubleRow matmul + exp/accum for column groups
                cgs over all 8 row-tiles. Stationary = own block (etn[0])."""
                wid = len(cgs) * CGW
                for t in range(T8):
                    ps = PS.tile([128, wid], f32, tag="mm", name="psmm")
                    for k2 in range(KT // 2):
                        ksl = slice(2 * k2, 2 * k2 + 2)
                        for ci, c in enumerate(cgs):
                            for n in range(CGW // 512):
                                lo = ci * CGW + n * 512
                                nc.tensor.matmul(
                                    ps[:, lo:lo + 512],
                                    etn[0][:, ksl, t * 128:(t + 1) * 128],
                                    etn[c][:, ksl, n * 512:(n + 1) * 512],
                                    start=(k2 == 0), stop=(k2 == KT // 2 - 1),
                                    perf_mode=DR)
                    sce = S.tile([128, wid], bf16, tag="expout", name="sce")
                    col = t * NG + gi
                    nc.scalar.activation(sce[:], ps[:], AF.Exp, scale=ASCALE,
                                         accum_out=rsums[:, col:col + 1])

            # ---- paced emission ----
            load_c(0)
            sumsq(0)
            rsqrt(rb8[:], ss64[:, 0:T8], T8)
            rchain(0, r8_src=rb8[:])
            load_c(1)
            sumsq(1)
            rchain(1)
            main_group(0, (0,))

            load_c(2)
            load_c(3)
            sumsq(2)
            rchain(2)
            sumsq(3)
            rchain(3)
            main_group(1, (1, 2))

            load_c(4)
            load_c(5)
            sumsq(4)
            rchain(4)
            # positives: own (group 0) x partner (group 4) row-dots
            for t in range(T8):
                sco = S.tile([128, DIM], fp8, tag="sttv", name="scop")
                nc.vector.scalar_tensor_tensor(
                    sco[:], erm[0][:, t * DIM:(t + 1) * DIM], 1.0,
                    erm[4][:, t * DIM:(t + 1) * DIM], ALU.mult, ALU.mult,
                    accum_out=rawpos[:, t:t + 1])
            rsqrt(rp8[:], ss64[:, 4 * T8:5 * T8], T8)
            pt0 = P.tile([128, T8], f32, name="pt0")
            nc.vector.tensor_mul(pt0[:], rawpos[:], rb8[:])
            pt1 = P.tile([128, T8], f32, name="pt1")
            nc.vector.tensor_mul(pt1[:], pt0[:], rp8[:])
            nc.vector.tensor_scalar_mul(pos8[:], pt1[:], 1.0 / TEMP)
            sumsq(5)
            rchain(5)
            main_group(2, (3, 4))

            load_c(6)
            load_c(7)
            sumsq(6)
            rchain(6)
            sumsq(7)
            rchain(7)
            main_group(3, (5, 6))
            main_group(4, (7,))

            # ---- finalize: den = rowsum - e^{1/T}; sum(log(den) - pos) ----
            den8 = P.tile([128, T8], f32, name="den8")
            nc.vector.tensor_reduce(
                den8[:], rsums[:].rearrange("p (t c) -> p t c", c=NG),
                X, ALU.add)
            den8b = P.tile([128, T8], f32, name="den8b")
            nc.vector.tensor_scalar_add(den8b[:], den8[:], -EXP_DIAG)
            logd = S.tile([128, T8], f32, tag="logd", name="logd")
            tlog = P.tile([128, 1], f32, name="tlog")
            nc.scalar.activation(logd[:], den8b[:], AF.Ln, accum_out=tlog[:])
            tpos = P.tile([128, 1], f32, name="tpos")
            nc.vector.tensor_reduce(tpos[:], pos8[:], X, ALU.add)
            lv = P.tile([128, 1], f32, name="lv")
            nc.vector.tensor_sub(lv[:], tlog[:], tpos[:])
            psf = PS.tile([1, 1], f32, tag="mm", name="psf")
            nc.tensor.matmul(psf[:], lv[:], ones[:], start=True, stop=True)
            ob = P.tile([1, 1], f32, name="ob")
            nc.vector.tensor_copy(ob[:], psf[:])
            nc.sync.dma_start(out_d[:], ob[:])

    nc.compile()
    return nc


def _get_nc():
    if "nc" not in _CACHE:
        _CACHE["nc"] = _build()
    return _CACHE["nc"]


def _in_maps(emb_i, emb_j):
    bf = ml_dtypes.bfloat16
    f8 = ml_dtypes.float8_e4m3
    E = np.concatenate([np.asarray(emb_i, dtype=np.float32),
                        np.asarray(emb_j, dtype=np.float32)], axis=0)
    Equ8 = E.astype(f8).view(np.uint8)                  # [8192, 512]
    ETu8 = np.ascontiguousarray(Equ8.T)                 # [512, 8192]
    SEL = np.zeros((128, T8 * 128), dtype=bf)
    for tp in range(T8):
        SEL[tp, tp * 128:(tp + 1) * 128] = ALPHA
    IDEN = np.eye(128, dtype=bf)
    maps = []
    for k in range(NCORES):
        s = k * RPC
        et_rot = np.ascontiguousarray(np.roll(ETu8, -s, axis=1))
        Er = np.roll(Equ8, -s, axis=0)
        ermr = np.ascontiguousarray(
            Er.reshape(B2 // 128, 128, DIM).transpose(1, 0, 2).reshape(128, -1))
        maps.append({
            "et": et_rot.view(f8),
            "erm": ermr.view(f8),
            "iden": IDEN,
            "sel": SEL,
        })
    return maps


def _run(emb_i, emb_j, trace=False):
    from concourse.bass_utils import run_bass_kernel_spmd
    nc = _get_nc()
    res = run_bass_kernel_spmd(nc, _in_maps(emb_i, emb_j),
                               list(range(NCORES)), trace=trace)
    total = sum(float(res.results[i]["out"][0, 0]) for i in range(NCORES))
    loss = np.float32(total / B2)
    return loss, res


def kernel(emb_i, emb_j):
    return _run(emb_i, emb_j, trace=False)[0]
